# revision 30
# baseline (speedup 1.0000x reference)
"""Trainium2 Bass kernel for pairwise-MLP GNN message passing.

Computation (per batch b, position l):
    x[i,j] = concat(states[l,i], states[l,j])           # [N,N,2D]
    out    = sigmoid(MLP(x))                            # [N,N,8], MLP: 32->64->64->8

Factorization used on device: the first linear layer splits into
A = states @ W1[:D] + b1 and B = states @ W1[D:], so
h1[i,j] = relu(A[i] + B[j]) — the N^2 expansion happens as a cheap
broadcast add on the vector engine instead of an N^2-row matmul.

Sharding: data-parallel over batch, core c <- batch c (8 cores, B=8).

Device layout (per core, L=64 l-blocks, 2 l-blocks = 1 "sb" superblock,
2 sbs = 1 "pair", 4 pairs = 1 DMA chunk):
  - features live on partitions: partitions 0:64 = even l-block of the sb,
    64:128 = odd l-block (via a host-side shifted copy of states^T feeding
    block-diagonal-packed matmuls).
  - pair columns col = 32*i + j, 1024 per l-block.
  - L2 runs as 4 concurrent 64x64x512 tile_position matmuls into one
    2-bank psum2; eviction split 768 cols on ScalarE / 256 on VectorE.
  - L3 runs as 8 concurrent 64x32x256 tile_position matmuls; psum3
    partition group 32g..32g+32 holds pair-column chunk g.
  - Sigmoid evicts psum3 as fp16 into a [128, 4096] tile covering 4
    pairs; one gathered DMA per chunk moves the 32 useful partitions.
"""

import os
import sys

import numpy as np

for _p in ("/opt/trn_rl_repo", "/root/.axon_site/_ro/trn_rl_repo"):
    if os.path.isdir(_p) and _p not in sys.path:
        sys.path.insert(0, _p)

from concourse import bacc, mybir, tile
from concourse.bass_utils import run_bass_kernel_spmd

B, L, N, D = 8, 64, 32, 16
H = 64            # hidden width (h1 and h2)
F = 8             # out_dim
NCORES = 8
NSB = L // 2      # 32 superblocks per core
NPAIR = NSB // 2  # 16 pairs per core
COLS = N * N      # 1024 pair columns per l-block
EV_SPLIT = 768    # h2 eviction: cols [0, EV_SPLIT) on ScalarE, rest on VectorE

FP32 = mybir.dt.float32
FP16 = mybir.dt.float16
OUT_DT = mybir.dt.float16  # device output dtype
BF16 = mybir.dt.bfloat16
NP_BF16 = mybir.dt.np(BF16)

_PROGRAM = None  # (nc, input_names)
LAST_RESULT = None  # BassKernelResults of the most recent kernel() call


def _build_program():
    nc = bacc.Bacc("TRN2", target_bir_lowering=False, debug=False)

    d_statesQ = nc.dram_tensor("statesQ", [16, 2048], BF16, kind="ExternalInput").ap()
    d_statesQs = nc.dram_tensor("statesQs", [16, 2048], BF16, kind="ExternalInput").ap()
    d_Wl1 = nc.dram_tensor("Wl1", [48, 128], BF16, kind="ExternalInput").ap()
    d_W23 = nc.dram_tensor("W23", [128, 96], BF16, kind="ExternalInput").ap()
    d_biases = nc.dram_tensor("biases", [128, 3], FP32, kind="ExternalInput").ap()
    # chunk t = pairs 4t..4t+4; [t, group g, feature f, col]
    d_out = nc.dram_tensor(
        "out", [NPAIR // 4, 4, F, 4 * COLS], OUT_DT, kind="ExternalOutput"
    ).ap()

    add = mybir.AluOpType.add
    max_ = mybir.AluOpType.max
    AF = mybir.ActivationFunctionType

    with tile.TileContext(nc) as tc:
        with tc.tile_pool(name="const", bufs=1) as const_pool:
            statesQ = const_pool.tile([64, 2048], BF16, name="statesQ_t")[:]
            Wl1 = const_pool.tile([48, 128], BF16, name="Wl1_t")[:]
            W23 = const_pool.tile([128, 96], BF16, name="W23_t")[:]
            biases = const_pool.tile([128, 3], FP32, name="biases_t")[:]
            W2q = W23[:, 0:64]
            W3q = W23[:, 64:96]
            bias1 = biases[:, 0:1]
            bias2 = biases[:, 1:2]
            bias3 = biases[:, 2:3]
            A2dup = const_pool.tile([128, 2 * COLS], BF16, name="A2dup_t")[:]
            B2s = const_pool.tile([128, COLS], BF16, name="B2s_t")[:]

            nc.sync.dma_start(out=statesQ[0:16], in_=d_statesQ)
            nc.sync.dma_start(out=statesQ[32:48], in_=d_statesQs)
            nc.sync.dma_start(out=Wl1, in_=d_Wl1)
            nc.sync.dma_start(out=W23, in_=d_W23)
            nc.sync.dma_start(out=biases, in_=d_biases)

            # ---- Layer 1: A2/B2 = per-agent halves of the first linear layer.
            # A2[p, 32*sb + i]: p<64 -> even l-block (2sb), p>=64 -> odd (2sb+1)
            # via the shifted rows 32:48 of statesQ.
            with tc.tile_pool(name="abps", bufs=1, space="PSUM") as ab_pool:
                A2ps = ab_pool.tile([128, COLS], FP32, tag="a2", name="A2ps_t")[:]
                B2ps = ab_pool.tile([128, COLS], FP32, tag="b2", name="B2ps_t")[:]
                rhs_even = statesQ[0:16].rearrange("p (s c) -> p s c", s=32)
                rhs_odd = statesQ[32:48].rearrange("p (s c) -> p s c", s=32)
                for w_lo, ps in ((0, A2ps), (64, B2ps)):
                    for half, rhs in ((0, rhs_even), (1, rhs_odd)):
                        lhsT = Wl1[32 * half : 32 * half + 16, w_lo : w_lo + 64]
                        for sbh in (0, 1):
                            nc.tensor.matmul(
                                ps[64 * half : 64 * half + 64, 512 * sbh : 512 * sbh + 512],
                                lhsT,
                                rhs[:, 16 * sbh : 16 * sbh + 16, 0:32],
                            )
                # Evict A2 twice (duplicated pairs so the later broadcast add
                # keeps an innermost unit stride), folding in b1; B2 plain.
                # Pair-0's slice is evicted first in small ops so the first
                # h1 ADD can start ~2.5us earlier.
                dupview = A2dup.rearrange("p (c two) -> p two c", two=2)
                nc.scalar.activation(dupview[:, 0, 0:64], A2ps[:, 0:64], AF.Identity, bias=bias1)
                nc.scalar.activation(dupview[:, 1, 0:64], A2ps[:, 0:64], AF.Identity, bias=bias1)
                nc.vector.tensor_copy(B2s[:, 0:64], B2ps[:, 0:64])
                nc.scalar.activation(dupview[:, 0, 64:1024], A2ps[:, 64:1024], AF.Identity, bias=bias1)
                nc.scalar.activation(dupview[:, 1, 64:1024], A2ps[:, 64:1024], AF.Identity, bias=bias1)
                nc.vector.tensor_copy(B2s[:, 64:1024], B2ps[:, 64:1024])

            with (
                tc.tile_pool(name="work", bufs=8) as work_pool,
                tc.tile_pool(name="sigp", bufs=2) as sig_pool,
                tc.tile_pool(name="l2ps", bufs=1, space="PSUM") as l2_pool,
                tc.tile_pool(name="l3ps", bufs=2, space="PSUM") as l3_pool,
            ):
                # Software pipeline, 1 sb deep: L3(sb) (which needs S2(sb)'s
                # eviction) is emitted after L2(sb+1) so it never blocks the
                # next L2 in the strict-FIFO PE queue.
                h2_tiles = {}     # pair -> (h2a AP, h2b AP)
                psum3_tiles = {}  # pair -> psum3 AP
                sig_tiles = {}    # chunk -> sig AP
                dma_backlog = []  # (chunk, sig AP, next group) pending DMAs

                def emit_s1_add(pair):
                    h1pre = work_pool.tile([128, 2 * COLS], BF16, tag="h1pre", name="h1pre_t")[:]
                    a_in = (
                        A2dup[:, 128 * pair : 128 * pair + 128]
                        .rearrange("p (s i two) -> p s i two", s=2, two=2)
                        .unsqueeze(3)
                        .broadcast_to([128, 2, 32, 16, 2])
                    )
                    b_in = (
                        B2s[:, 64 * pair : 64 * pair + 64]
                        .rearrange("p (s jh jl) -> p s jh jl", s=2, jl=2)
                        .unsqueeze(2)
                        .broadcast_to([128, 2, 32, 16, 2])
                    )
                    h1pre_v = h1pre.rearrange(
                        "p (s i jh jl) -> p s i jh jl", s=2, i=32, jl=2
                    )
                    nc.vector.tensor_add(h1pre_v, a_in, b_in)
                    return h1pre

                def emit_s1_max(h1pre):
                    h1 = work_pool.tile([128, 2 * COLS], BF16, tag="h1", name="h1_t")[:]
                    nc.vector.tensor_scalar_max(h1, h1pre, 0.0)
                    return h1

                def emit_l2_pair(p, h1):
                    # Both sbs of the pair into ONE 4-bank psum (cols
                    # 1024*k + c).  4 concurrent quadrant matmuls per sb;
                    # col half 512:1024 has its partition halves swapped
                    # (even block on 64:128) so all 4 quadrants run at once.
                    # One strided mega-eviction per engine covers both sbs,
                    # halving the per-op fixed overhead on the pacer (ACT).
                    psum2 = l2_pool.tile([128, 2048], FP32, tag="l2", name="psum2_t")[:]
                    for k in (0, 1):
                        hk = h1[:, COLS * k : COLS * k + COLS]
                        ck = 1024 * k
                        nc.tensor.matmul(psum2[0:64, ck : ck + 512], W2q[0:64], hk[0:64, 0:512], tile_position=(0, 0))
                        nc.tensor.matmul(psum2[64:128, ck : ck + 512], W2q[64:128], hk[64:128, 0:512], tile_position=(64, 64))
                        nc.tensor.matmul(psum2[64:128, ck + 512 : ck + 1024], W2q[0:64], hk[0:64, 512:1024], tile_position=(0, 64))
                        nc.tensor.matmul(psum2[0:64, ck + 512 : ck + 1024], W2q[64:128], hk[64:128, 512:1024], tile_position=(64, 0))
                    h2a = work_pool.tile([128, 2 * EV_SPLIT], BF16, tag="h2a", name="h2a_t")[:]
                    h2b = work_pool.tile([128, 2 * (COLS - EV_SPLIT)], BF16, tag="h2b", name="h2b_t")[:]
                    ps_v = psum2.rearrange("p (k c) -> p k c", k=2)
                    nc.scalar.activation(
                        h2a.rearrange("p (k c) -> p k c", k=2),
                        ps_v[:, :, 0:EV_SPLIT], AF.Relu, bias=bias2,
                    )
                    nc.vector.tensor_scalar(
                        h2b.rearrange("p (k c) -> p k c", k=2),
                        ps_v[:, :, EV_SPLIT:COLS], bias2, 0.0, add, max_,
                    )
                    h2_tiles[p] = (h2a, h2b)

                def emit_l3(sb):
                    pair, k = divmod(sb, 2)
                    if k == 0:
                        psum3_tiles[pair] = l3_pool.tile([128, COLS], FP32, tag="l3", name="psum3_t")[:]
                    psum3 = psum3_tiles[pair]
                    h2a, h2b = h2_tiles[pair]
                    if k == 1:
                        del h2_tiles[pair]
                    # 8-tile L3: 256-col streams; psum3 col = 512p + 256k + q
                    # so col-group partner tiles (parities) land in different
                    # PSUM banks (concurrent same-bank writes from one
                    # col-group are a HW collision).
                    for g in range(4):
                        for p in (0, 1):
                            r = 64 * p if g < 2 else 64 * (1 - p)
                            c0 = 256 * g
                            if c0 >= EV_SPLIT:
                                cb = (COLS - EV_SPLIT) * k + c0 - EV_SPLIT
                                rhs = h2b[r : r + 64, cb : cb + 256]
                            else:
                                rhs = h2a[r : r + 64, EV_SPLIT * k + c0 : EV_SPLIT * k + c0 + 256]
                            nc.tensor.matmul(
                                psum3[32 * g : 32 * g + 32, 512 * p + 256 * k : 512 * p + 256 * k + 256],
                                W3q[r : r + 64],
                                rhs,
                                tile_position=(r, 32 * g),
                            )

                def emit_sigmoid_dma(pair):
                    t, u = divmod(pair, 4)
                    if u == 0:
                        sig_tiles[t] = sig_pool.tile([128, 4 * COLS], OUT_DT, tag="sig", name="sig_t")[:]
                    sig4 = sig_tiles[t]
                    psum3 = psum3_tiles.pop(pair)
                    nc.scalar.activation(
                        sig4[:, COLS * u : COLS * u + COLS], psum3, AF.Sigmoid, bias=bias3
                    )
                    if pair == NPAIR - 1:
                        # Final chunk: 2 DMAs on the sync HWDGE ring, 2 on
                        # the scalar ring (idle after the last sigmoid) so
                        # descriptor gen + drain run in parallel.
                        for g in range(4):
                            eng = nc.sync if g < 2 else nc.scalar
                            eng.dma_start(
                                out=d_out[t, g], in_=sig4[32 * g : 32 * g + F, :]
                            )
                    elif u == 3:
                        dma_backlog.append((t, sig4, 0))

                def emit_chunk_dma_piece():
                    # De-burst: one group-DMA of the previous chunk per pair.
                    if dma_backlog:
                        t, sig4, g = dma_backlog[0]
                        nc.sync.dma_start(
                            out=d_out[t, g], in_=sig4[32 * g : 32 * g + F, :]
                        )
                        if g == 3:
                            dma_backlog.pop(0)
                        else:
                            dma_backlog[0] = (t, sig4, g + 1)

                # Pair-level batching: both L2 groups back-to-back (W2 loads
                # amortized), then both L3 groups of the PREVIOUS pair (their
                # evictions finished a full cycle ago, so the strict-FIFO PE
                # queue never stalls and W3 streams back-to-back).  DVE order
                # per pair: ADDMAX(2p), ADDMAX(2p+1), ADD(p+1), MAX(p+1) —
                # evictions first so psum2 recycles early.
                h1_cur = emit_s1_max(emit_s1_add(0))
                for p in range(NPAIR):
                    emit_l2_pair(p, h1_cur)
                    if p >= 1:
                        emit_l3(2 * p - 2)
                        emit_l3(2 * p - 1)
                        emit_sigmoid_dma(p - 1)
                        emit_chunk_dma_piece()
                    if p + 1 < NPAIR:
                        h1_cur = emit_s1_max(emit_s1_add(p + 1))
                emit_l3(NSB - 2)
                emit_l3(NSB - 1)
                emit_sigmoid_dma(NPAIR - 1)

    nc.compile()
    input_names = ["statesQ", "Wl1", "W23", "biases"]
    return nc, input_names


def get_program():
    global _PROGRAM
    if _PROGRAM is None:
        _PROGRAM = _build_program()
    return _PROGRAM


def make_inputs(states, W1, b1, W2, b2, W3, b3):
    """Host-side prep: per-core statesQ + shared packed weights/biases."""
    states = np.asarray(states, np.float32)
    W1 = np.asarray(W1, np.float32)
    W2 = np.asarray(W2, np.float32)
    W3 = np.asarray(W3, np.float32)
    b1 = np.asarray(b1, np.float32)
    b2 = np.asarray(b2, np.float32)
    b3 = np.asarray(b3, np.float32)

    Wl1 = np.zeros((48, 128), NP_BF16)
    Wl1[0:16, 0:64] = W1[:D].astype(NP_BF16)
    Wl1[0:16, 64:128] = W1[D:].astype(NP_BF16)
    Wl1[32:48, 0:64] = W1[:D].astype(NP_BF16)
    Wl1[32:48, 64:128] = W1[D:].astype(NP_BF16)

    W23 = np.zeros((128, 96), NP_BF16)
    W23[0:64, 0:64] = W2.astype(NP_BF16)
    W23[64:128, 0:64] = W2.astype(NP_BF16)
    W23[0:64, 64:72] = W3.astype(NP_BF16)
    W23[64:128, 64:72] = W3.astype(NP_BF16)

    biases = np.zeros((128, 3), np.float32)
    biases[:, 0] = np.tile(b1, 2)
    biases[:, 1] = np.tile(b2, 2)
    biases[:, 2] = np.tile(np.concatenate([b3, np.zeros(24, np.float32)]), 4)

    shared = {"Wl1": Wl1, "W23": W23, "biases": biases}

    in_maps = []
    for c in range(NCORES):
        # statesT[d, 32*l + i] = states[c, l, i, d]
        statesT = states[c].reshape(L * N, D).T.astype(NP_BF16)
        sQs = np.zeros((16, 2048), NP_BF16)
        sQs[:, : 2048 - 32] = statesT[:, 32:]
        in_maps.append({"statesQ": statesT.copy(), "statesQs": sQs, **shared})
    return in_maps


def decode_output(raw):
    """Invert the device output layout -> [L, N, N, F] for one core.

    raw: [4, 4, F, 4096] fp16 = [chunk t, group g, f, col]; col =
    1024u + 512k + 256p + q with pair = 4t+u, sb = 2*pair+k,
    l = 2*sb + p = 16t + 4u + 2k + p; pair-col = 256g + q = 32i + j.
    """
    ov = raw.reshape(4, 4, F, 4, 2, 2, 8, 32)        # [t, g, f, u, p, k, qi, j]
    ov = ov.transpose(0, 3, 5, 4, 1, 6, 7, 2)        # [t, u, k, p, g, qi, j, f]
    return np.ascontiguousarray(ov.reshape(L, N, N, F).astype(np.float32))


def _ensure_ntff_hook():
    """Best-effort shim for the missing antenv.axon_hooks module so
    run_bass_kernel_spmd(trace=True) can capture NTFF profiles under axon."""
    import types

    try:
        from antenv.axon_hooks import get_axon_ntff_profile_hook  # noqa: F401
        return
    except ImportError:
        pass
    try:
        if "/root/.axon_site" not in sys.path:
            sys.path.insert(0, "/root/.axon_site")
        from trn_agent_boot.trn_boot import _ntff_profile_via_ctypes

        hook = _ntff_profile_via_ctypes("/opt/axon/libaxon_pjrt.so")
        import antenv

        mod = types.ModuleType("antenv.axon_hooks")
        mod._hook = hook
        mod.set_axon_ntff_profile_hook = lambda h: setattr(mod, "_hook", h)
        mod.get_axon_ntff_profile_hook = lambda: mod._hook
        sys.modules["antenv.axon_hooks"] = mod
        antenv.axon_hooks = mod
    except Exception as e:  # tracing is optional; never break the run
        print(f"ntff hook shim failed: {e}", file=sys.stderr)


def kernel(states, W1, b1, W2, b2, W3, b3):
    global LAST_RESULT
    nc, _ = get_program()
    if os.environ.get("KERNEL_TRACE"):
        _ensure_ntff_hook()
    in_maps = make_inputs(states, W1, b1, W2, b2, W3, b3)
    res = run_bass_kernel_spmd(
        nc,
        in_maps,
        core_ids=list(range(NCORES)),
        trace=bool(os.environ.get("KERNEL_TRACE")),
    )
    LAST_RESULT = res
    out = np.empty((B, L, N, N, F), np.float32)
    for c in range(NCORES):
        out[c] = decode_output(res.results[c]["out"])
    return out


# revision 31
# speedup vs baseline: 1.4655x; 1.4655x over previous
"""Trainium2 Bass kernel for pairwise-MLP GNN message passing.

Computation (per batch b, position l):
    x[i,j] = concat(states[l,i], states[l,j])           # [N,N,2D]
    out    = sigmoid(MLP(x))                            # [N,N,8], MLP: 32->64->64->8

Factorization used on device: the first linear layer splits into
A = states @ W1[:D] + b1 and B = states @ W1[D:], so
h1[i,j] = relu(A[i] + B[j]) — the N^2 expansion happens as a cheap
broadcast add on the vector engine instead of an N^2-row matmul.

Sharding: data-parallel over batch, core c <- batch c (8 cores, B=8).

Device layout (per core, L=64 l-blocks, 2 l-blocks = 1 "sb" superblock,
2 sbs = 1 "pair", 4 pairs = 1 DMA chunk):
  - features live on partitions: partitions 0:64 = even l-block of the sb,
    64:128 = odd l-block (via a host-side shifted copy of states^T feeding
    block-diagonal-packed matmuls).
  - pair columns col = 32*i + j, 1024 per l-block.
  - L2 runs as 4 concurrent 64x64x512 tile_position matmuls into one
    2-bank psum2; eviction split 768 cols on ScalarE / 256 on VectorE.
  - L3 runs as 8 concurrent 64x32x256 tile_position matmuls; psum3
    partition group 32g..32g+32 holds pair-column chunk g.
  - Sigmoid evicts psum3 as fp16 into a [128, 4096] tile covering 4
    pairs; one gathered DMA per chunk moves the 32 useful partitions.
"""

import os
import sys

import numpy as np

for _p in ("/opt/trn_rl_repo", "/root/.axon_site/_ro/trn_rl_repo"):
    if os.path.isdir(_p) and _p not in sys.path:
        sys.path.insert(0, _p)

from concourse import bacc, mybir, tile
from concourse.bass_utils import run_bass_kernel_spmd

B, L, N, D = 8, 64, 32, 16
H = 64            # hidden width (h1 and h2)
F = 8             # out_dim
NCORES = 8
NSB = L // 2      # 32 superblocks per core
NPAIR = NSB // 2  # 16 pairs per core
COLS = N * N      # 1024 pair columns per l-block
EV_SPLIT = 768    # h2 eviction: cols [0, EV_SPLIT) on ScalarE, rest on VectorE

FP32 = mybir.dt.float32
FP16 = mybir.dt.float16
OUT_DT = mybir.dt.float16  # device output dtype
BF16 = mybir.dt.bfloat16
NP_BF16 = mybir.dt.np(BF16)

_PROGRAM = None  # (nc, input_names)
LAST_RESULT = None  # BassKernelResults of the most recent kernel() call


def _build_program():
    nc = bacc.Bacc("TRN2", target_bir_lowering=False, debug=False)

    d_statesQ = nc.dram_tensor("statesQ", [16, 2048], BF16, kind="ExternalInput").ap()
    d_statesQs = nc.dram_tensor("statesQs", [16, 2048], BF16, kind="ExternalInput").ap()
    d_Wl1 = nc.dram_tensor("Wl1", [48, 128], BF16, kind="ExternalInput").ap()
    d_W23 = nc.dram_tensor("W23", [128, 96], BF16, kind="ExternalInput").ap()
    d_biases = nc.dram_tensor("biases", [128, 3], FP32, kind="ExternalInput").ap()
    # chunk t = pairs 4t..4t+4; [t, group g, feature f, col]
    d_out = nc.dram_tensor(
        "out", [NPAIR // 4, 4, F, 4 * COLS], OUT_DT, kind="ExternalOutput"
    ).ap()

    add = mybir.AluOpType.add
    max_ = mybir.AluOpType.max
    AF = mybir.ActivationFunctionType

    with tile.TileContext(nc) as tc:
        with tc.tile_pool(name="const", bufs=1) as const_pool:
            statesQ = const_pool.tile([64, 2048], BF16, name="statesQ_t")[:]
            Wl1 = const_pool.tile([48, 128], BF16, name="Wl1_t")[:]
            W23 = const_pool.tile([128, 96], BF16, name="W23_t")[:]
            biases = const_pool.tile([128, 3], FP32, name="biases_t")[:]
            W2q = W23[:, 0:64]
            W3q = W23[:, 64:96]
            bias1 = biases[:, 0:1]
            bias2 = biases[:, 1:2]
            bias3 = biases[:, 2:3]
            A2dup = const_pool.tile([128, 2 * COLS], BF16, name="A2dup_t")[:]
            B2s = const_pool.tile([128, COLS], BF16, name="B2s_t")[:]

            nc.sync.dma_start(out=statesQ[0:16], in_=d_statesQ)
            nc.sync.dma_start(out=statesQ[32:48], in_=d_statesQs)
            nc.sync.dma_start(out=Wl1, in_=d_Wl1)
            nc.sync.dma_start(out=W23, in_=d_W23)
            nc.sync.dma_start(out=biases, in_=d_biases)

            # ---- Layer 1: A2/B2 = per-agent halves of the first linear layer.
            # A2[p, 32*sb + i]: p<64 -> even l-block (2sb), p>=64 -> odd (2sb+1)
            # via the shifted rows 32:48 of statesQ.
            with tc.tile_pool(name="abps", bufs=1, space="PSUM") as ab_pool:
                A2ps = ab_pool.tile([128, COLS], FP32, tag="a2", name="A2ps_t")[:]
                B2ps = ab_pool.tile([128, COLS], FP32, tag="b2", name="B2ps_t")[:]
                rhs_even = statesQ[0:16].rearrange("p (s c) -> p s c", s=32)
                rhs_odd = statesQ[32:48].rearrange("p (s c) -> p s c", s=32)
                for w_lo, ps in ((0, A2ps), (64, B2ps)):
                    for half, rhs in ((0, rhs_even), (1, rhs_odd)):
                        lhsT = Wl1[32 * half : 32 * half + 16, w_lo : w_lo + 64]
                        for sbh in (0, 1):
                            nc.tensor.matmul(
                                ps[64 * half : 64 * half + 64, 512 * sbh : 512 * sbh + 512],
                                lhsT,
                                rhs[:, 16 * sbh : 16 * sbh + 16, 0:32],
                            )
                # Evict A2 twice (duplicated pairs so the later broadcast add
                # keeps an innermost unit stride), folding in b1; B2 plain.
                # Pair-0's slice is evicted first in small ops so the first
                # h1 ADD can start ~2.5us earlier.
                dupview = A2dup.rearrange("p (c two) -> p two c", two=2)
                nc.scalar.activation(dupview[:, 0, 0:64], A2ps[:, 0:64], AF.Identity, bias=bias1)
                nc.scalar.activation(dupview[:, 1, 0:64], A2ps[:, 0:64], AF.Identity, bias=bias1)
                nc.vector.tensor_copy(B2s[:, 0:64], B2ps[:, 0:64])
                nc.scalar.activation(dupview[:, 0, 64:1024], A2ps[:, 64:1024], AF.Identity, bias=bias1)
                nc.scalar.activation(dupview[:, 1, 64:1024], A2ps[:, 64:1024], AF.Identity, bias=bias1)
                nc.vector.tensor_copy(B2s[:, 64:1024], B2ps[:, 64:1024])

            with (
                tc.tile_pool(name="work", bufs=8) as work_pool,
                tc.tile_pool(name="sigp", bufs=2) as sig_pool,
                tc.tile_pool(name="l2ps", bufs=2, space="PSUM") as l2_pool,
                tc.tile_pool(name="l3ps", bufs=2, space="PSUM") as l3_pool,
            ):
                # Software pipeline, 1 sb deep: L3(sb) (which needs S2(sb)'s
                # eviction) is emitted after L2(sb+1) so it never blocks the
                # next L2 in the strict-FIFO PE queue.
                h2_tiles = {}     # pair -> (h2a AP, h2b AP)
                psum3_tiles = {}  # pair -> psum3 AP
                sig_tiles = {}    # chunk -> sig AP
                dma_backlog = []  # (chunk, sig AP, next group) pending DMAs

                def emit_s1_add(pair):
                    h1pre = work_pool.tile([128, 2 * COLS], BF16, tag="h1pre", name="h1pre_t")[:]
                    a_in = (
                        A2dup[:, 128 * pair : 128 * pair + 128]
                        .rearrange("p (s i two) -> p s i two", s=2, two=2)
                        .unsqueeze(3)
                        .broadcast_to([128, 2, 32, 16, 2])
                    )
                    b_in = (
                        B2s[:, 64 * pair : 64 * pair + 64]
                        .rearrange("p (s jh jl) -> p s jh jl", s=2, jl=2)
                        .unsqueeze(2)
                        .broadcast_to([128, 2, 32, 16, 2])
                    )
                    h1pre_v = h1pre.rearrange(
                        "p (s i jh jl) -> p s i jh jl", s=2, i=32, jl=2
                    )
                    nc.vector.tensor_add(h1pre_v, a_in, b_in)
                    return h1pre

                def emit_s1_max(h1pre):
                    h1 = work_pool.tile([128, 2 * COLS], BF16, tag="h1", name="h1_t")[:]
                    nc.vector.tensor_scalar_max(h1, h1pre, 0.0)
                    return h1

                def emit_l2_pair(p, h1):
                    # Per-sb psums (bufs=2) keep the PE pipelined one sb
                    # ahead of the evictions.  Col half 512:1024 has its
                    # partition halves swapped (even block on 64:128) so all
                    # 4 quadrants run at once.
                    h2a = work_pool.tile([128, 2 * EV_SPLIT], BF16, tag="h2a", name="h2a_t")[:]
                    h2b = work_pool.tile([128, 2 * (COLS - EV_SPLIT)], BF16, tag="h2b", name="h2b_t")[:]
                    for k in (0, 1):
                        hk = h1[:, COLS * k : COLS * k + COLS]
                        psum2 = l2_pool.tile([128, 1024], FP32, tag="l2", name="psum2_t")[:]
                        nc.tensor.matmul(psum2[0:64, 0:512], W2q[0:64], hk[0:64, 0:512], tile_position=(0, 0))
                        nc.tensor.matmul(psum2[64:128, 0:512], W2q[64:128], hk[64:128, 0:512], tile_position=(64, 64))
                        nc.tensor.matmul(psum2[64:128, 512:1024], W2q[0:64], hk[0:64, 512:1024], tile_position=(0, 64))
                        nc.tensor.matmul(psum2[0:64, 512:1024], W2q[64:128], hk[64:128, 512:1024], tile_position=(64, 0))
                        nc.scalar.activation(
                            h2a[:, EV_SPLIT * k : EV_SPLIT * k + EV_SPLIT],
                            psum2[:, 0:EV_SPLIT], AF.Relu, bias=bias2,
                        )
                        nc.vector.tensor_scalar(
                            h2b[:, (COLS - EV_SPLIT) * k : (COLS - EV_SPLIT) * (k + 1)],
                            psum2[:, EV_SPLIT:COLS], bias2, 0.0, add, max_,
                        )
                    h2_tiles[p] = (h2a, h2b)

                def emit_l3(sb):
                    pair, k = divmod(sb, 2)
                    if k == 0:
                        psum3_tiles[pair] = l3_pool.tile([128, COLS], FP32, tag="l3", name="psum3_t")[:]
                    psum3 = psum3_tiles[pair]
                    h2a, h2b = h2_tiles[pair]
                    if k == 1:
                        del h2_tiles[pair]
                    # 8-tile L3: 256-col streams; psum3 col = 512p + 256k + q
                    # so col-group partner tiles (parities) land in different
                    # PSUM banks (concurrent same-bank writes from one
                    # col-group are a HW collision).
                    for g in range(4):
                        for p in (0, 1):
                            r = 64 * p if g < 2 else 64 * (1 - p)
                            c0 = 256 * g
                            if c0 >= EV_SPLIT:
                                cb = (COLS - EV_SPLIT) * k + c0 - EV_SPLIT
                                rhs = h2b[r : r + 64, cb : cb + 256]
                            else:
                                rhs = h2a[r : r + 64, EV_SPLIT * k + c0 : EV_SPLIT * k + c0 + 256]
                            nc.tensor.matmul(
                                psum3[32 * g : 32 * g + 32, 512 * p + 256 * k : 512 * p + 256 * k + 256],
                                W3q[r : r + 64],
                                rhs,
                                tile_position=(r, 32 * g),
                            )

                def emit_sigmoid_dma(pair):
                    t, u = divmod(pair, 4)
                    if u == 0:
                        sig_tiles[t] = sig_pool.tile([128, 4 * COLS], OUT_DT, tag="sig", name="sig_t")[:]
                    sig4 = sig_tiles[t]
                    psum3 = psum3_tiles.pop(pair)
                    nc.scalar.activation(
                        sig4[:, COLS * u : COLS * u + COLS], psum3, AF.Sigmoid, bias=bias3
                    )
                    if pair == NPAIR - 1:
                        # Final chunk: 2 DMAs on the sync HWDGE ring, 2 on
                        # the scalar ring (idle after the last sigmoid) so
                        # descriptor gen + drain run in parallel.
                        for g in range(4):
                            eng = nc.sync if g < 2 else nc.scalar
                            eng.dma_start(
                                out=d_out[t, g], in_=sig4[32 * g : 32 * g + F, :]
                            )
                    elif u == 3:
                        dma_backlog.append((t, sig4, 0))

                def emit_chunk_dma_piece():
                    # De-burst: one group-DMA of the previous chunk per pair.
                    if dma_backlog:
                        t, sig4, g = dma_backlog[0]
                        nc.sync.dma_start(
                            out=d_out[t, g], in_=sig4[32 * g : 32 * g + F, :]
                        )
                        if g == 3:
                            dma_backlog.pop(0)
                        else:
                            dma_backlog[0] = (t, sig4, g + 1)

                # Pair-level batching: both L2 groups back-to-back (W2 loads
                # amortized), then both L3 groups of the PREVIOUS pair (their
                # evictions finished a full cycle ago, so the strict-FIFO PE
                # queue never stalls and W3 streams back-to-back).  DVE order
                # per pair: ADDMAX(2p), ADDMAX(2p+1), ADD(p+1), MAX(p+1) —
                # evictions first so psum2 recycles early.
                h1_cur = emit_s1_max(emit_s1_add(0))
                for p in range(NPAIR):
                    emit_l2_pair(p, h1_cur)
                    if p >= 1:
                        emit_l3(2 * p - 2)
                        emit_l3(2 * p - 1)
                        emit_sigmoid_dma(p - 1)
                        emit_chunk_dma_piece()
                    if p + 1 < NPAIR:
                        h1_cur = emit_s1_max(emit_s1_add(p + 1))
                emit_l3(NSB - 2)
                emit_l3(NSB - 1)
                emit_sigmoid_dma(NPAIR - 1)

    nc.compile()
    input_names = ["statesQ", "Wl1", "W23", "biases"]
    return nc, input_names


def get_program():
    global _PROGRAM
    if _PROGRAM is None:
        _PROGRAM = _build_program()
    return _PROGRAM


def make_inputs(states, W1, b1, W2, b2, W3, b3):
    """Host-side prep: per-core statesQ + shared packed weights/biases."""
    states = np.asarray(states, np.float32)
    W1 = np.asarray(W1, np.float32)
    W2 = np.asarray(W2, np.float32)
    W3 = np.asarray(W3, np.float32)
    b1 = np.asarray(b1, np.float32)
    b2 = np.asarray(b2, np.float32)
    b3 = np.asarray(b3, np.float32)

    Wl1 = np.zeros((48, 128), NP_BF16)
    Wl1[0:16, 0:64] = W1[:D].astype(NP_BF16)
    Wl1[0:16, 64:128] = W1[D:].astype(NP_BF16)
    Wl1[32:48, 0:64] = W1[:D].astype(NP_BF16)
    Wl1[32:48, 64:128] = W1[D:].astype(NP_BF16)

    W23 = np.zeros((128, 96), NP_BF16)
    W23[0:64, 0:64] = W2.astype(NP_BF16)
    W23[64:128, 0:64] = W2.astype(NP_BF16)
    W23[0:64, 64:72] = W3.astype(NP_BF16)
    W23[64:128, 64:72] = W3.astype(NP_BF16)

    biases = np.zeros((128, 3), np.float32)
    biases[:, 0] = np.tile(b1, 2)
    biases[:, 1] = np.tile(b2, 2)
    biases[:, 2] = np.tile(np.concatenate([b3, np.zeros(24, np.float32)]), 4)

    shared = {"Wl1": Wl1, "W23": W23, "biases": biases}

    in_maps = []
    for c in range(NCORES):
        # statesT[d, 32*l + i] = states[c, l, i, d]
        statesT = states[c].reshape(L * N, D).T.astype(NP_BF16)
        sQs = np.zeros((16, 2048), NP_BF16)
        sQs[:, : 2048 - 32] = statesT[:, 32:]
        in_maps.append({"statesQ": statesT.copy(), "statesQs": sQs, **shared})
    return in_maps


def decode_output(raw):
    """Invert the device output layout -> [L, N, N, F] for one core.

    raw: [4, 4, F, 4096] fp16 = [chunk t, group g, f, col]; col =
    1024u + 512k + 256p + q with pair = 4t+u, sb = 2*pair+k,
    l = 2*sb + p = 16t + 4u + 2k + p; pair-col = 256g + q = 32i + j.
    """
    ov = raw.reshape(4, 4, F, 4, 2, 2, 8, 32)        # [t, g, f, u, p, k, qi, j]
    ov = ov.transpose(0, 3, 5, 4, 1, 6, 7, 2)        # [t, u, k, p, g, qi, j, f]
    return np.ascontiguousarray(ov.reshape(L, N, N, F).astype(np.float32))


def _ensure_ntff_hook():
    """Best-effort shim for the missing antenv.axon_hooks module so
    run_bass_kernel_spmd(trace=True) can capture NTFF profiles under axon."""
    import types

    try:
        from antenv.axon_hooks import get_axon_ntff_profile_hook  # noqa: F401
        return
    except ImportError:
        pass
    try:
        if "/root/.axon_site" not in sys.path:
            sys.path.insert(0, "/root/.axon_site")
        from trn_agent_boot.trn_boot import _ntff_profile_via_ctypes

        hook = _ntff_profile_via_ctypes("/opt/axon/libaxon_pjrt.so")
        import antenv

        mod = types.ModuleType("antenv.axon_hooks")
        mod._hook = hook
        mod.set_axon_ntff_profile_hook = lambda h: setattr(mod, "_hook", h)
        mod.get_axon_ntff_profile_hook = lambda: mod._hook
        sys.modules["antenv.axon_hooks"] = mod
        antenv.axon_hooks = mod
    except Exception as e:  # tracing is optional; never break the run
        print(f"ntff hook shim failed: {e}", file=sys.stderr)


def kernel(states, W1, b1, W2, b2, W3, b3):
    global LAST_RESULT
    nc, _ = get_program()
    if os.environ.get("KERNEL_TRACE"):
        _ensure_ntff_hook()
    in_maps = make_inputs(states, W1, b1, W2, b2, W3, b3)
    res = run_bass_kernel_spmd(
        nc,
        in_maps,
        core_ids=list(range(NCORES)),
        trace=bool(os.environ.get("KERNEL_TRACE")),
    )
    LAST_RESULT = res
    out = np.empty((B, L, N, N, F), np.float32)
    for c in range(NCORES):
        out[c] = decode_output(res.results[c]["out"])
    return out


# revision 34
# speedup vs baseline: 1.4885x; 1.0157x over previous
"""Trainium2 Bass kernel for pairwise-MLP GNN message passing.

Computation (per batch b, position l):
    x[i,j] = concat(states[l,i], states[l,j])           # [N,N,2D]
    out    = sigmoid(MLP(x))                            # [N,N,8], MLP: 32->64->64->8

Factorization used on device: the first linear layer splits into
A = states @ W1[:D] + b1 and B = states @ W1[D:], so
h1[i,j] = relu(A[i] + B[j]) — the N^2 expansion happens as a cheap
broadcast add on the vector engine instead of an N^2-row matmul.

Sharding: data-parallel over batch, core c <- batch c (8 cores, B=8).

Device layout (per core, L=64 l-blocks, 2 l-blocks = 1 "sb" superblock,
2 sbs = 1 "pair", 4 pairs = 1 DMA chunk):
  - features live on partitions: partitions 0:64 = even l-block of the sb,
    64:128 = odd l-block (via a host-side shifted copy of states^T feeding
    block-diagonal-packed matmuls).
  - pair columns col = 32*i + j, 1024 per l-block.
  - L2 runs as 4 concurrent 64x64x512 tile_position matmuls into one
    2-bank psum2; eviction split 768 cols on ScalarE / 256 on VectorE.
  - L3 runs as 8 concurrent 64x32x256 tile_position matmuls; psum3
    partition group 32g..32g+32 holds pair-column chunk g.
  - Sigmoid evicts psum3 as fp16 into a [128, 4096] tile covering 4
    pairs; one gathered DMA per chunk moves the 32 useful partitions.
"""

import os
import sys

import numpy as np

for _p in ("/opt/trn_rl_repo", "/root/.axon_site/_ro/trn_rl_repo"):
    if os.path.isdir(_p) and _p not in sys.path:
        sys.path.insert(0, _p)

from concourse import bacc, mybir, tile
from concourse.bass_utils import run_bass_kernel_spmd

B, L, N, D = 8, 64, 32, 16
H = 64            # hidden width (h1 and h2)
F = 8             # out_dim
NCORES = 8
NSB = L // 2      # 32 superblocks per core
NPAIR = NSB // 2  # 16 pairs per core
COLS = N * N      # 1024 pair columns per l-block
EV_SPLIT = 768    # h2 eviction: cols [0, EV_SPLIT) on ScalarE, rest on VectorE

FP32 = mybir.dt.float32
FP16 = mybir.dt.float16
OUT_DT = mybir.dt.float16  # device output dtype
BF16 = mybir.dt.bfloat16
NP_BF16 = mybir.dt.np(BF16)

_PROGRAM = None  # (nc, input_names)
LAST_RESULT = None  # BassKernelResults of the most recent kernel() call


def _build_program():
    nc = bacc.Bacc("TRN2", target_bir_lowering=False, debug=False)

    d_statesQ = nc.dram_tensor("statesQ", [16, 2048], BF16, kind="ExternalInput").ap()
    d_statesQs = nc.dram_tensor("statesQs", [16, 2048], BF16, kind="ExternalInput").ap()
    d_Wl1 = nc.dram_tensor("Wl1", [48, 128], BF16, kind="ExternalInput").ap()
    d_W23 = nc.dram_tensor("W23", [128, 96], BF16, kind="ExternalInput").ap()
    d_biases = nc.dram_tensor("biases", [128, 3], FP32, kind="ExternalInput").ap()
    # chunk t = pairs 4t..4t+4; [t, group g, feature f, col]
    d_out = nc.dram_tensor(
        "out", [NPAIR // 4, 4, F, 4 * COLS], OUT_DT, kind="ExternalOutput"
    ).ap()

    add = mybir.AluOpType.add
    max_ = mybir.AluOpType.max
    AF = mybir.ActivationFunctionType

    with tile.TileContext(nc) as tc:
        with tc.tile_pool(name="const", bufs=1) as const_pool:
            statesQ = const_pool.tile([64, 2048], BF16, name="statesQ_t")[:]
            Wl1 = const_pool.tile([48, 128], BF16, name="Wl1_t")[:]
            W23 = const_pool.tile([128, 96], BF16, name="W23_t")[:]
            biases = const_pool.tile([128, 3], FP32, name="biases_t")[:]
            W2q = W23[:, 0:64]
            W3q = W23[:, 64:96]
            bias1 = biases[:, 0:1]
            bias2 = biases[:, 1:2]
            bias3 = biases[:, 2:3]
            A2dup = const_pool.tile([128, 2 * COLS], BF16, name="A2dup_t")[:]
            B2s = const_pool.tile([128, COLS], BF16, name="B2s_t")[:]

            nc.sync.dma_start(out=statesQ[0:16], in_=d_statesQ)
            nc.sync.dma_start(out=statesQ[32:48], in_=d_statesQs)
            nc.sync.dma_start(out=Wl1, in_=d_Wl1)
            nc.sync.dma_start(out=W23, in_=d_W23)
            nc.sync.dma_start(out=biases, in_=d_biases)

            # ---- Layer 1: A2/B2 = per-agent halves of the first linear layer.
            # A2[p, 32*sb + i]: p<64 -> even l-block (2sb), p>=64 -> odd (2sb+1)
            # via the shifted rows 32:48 of statesQ.
            with tc.tile_pool(name="abps", bufs=1, space="PSUM") as ab_pool:
                A2ps = ab_pool.tile([128, COLS], FP32, tag="a2", name="A2ps_t")[:]
                B2ps = ab_pool.tile([128, COLS], FP32, tag="b2", name="B2ps_t")[:]
                rhs_even = statesQ[0:16].rearrange("p (s c) -> p s c", s=32)
                rhs_odd = statesQ[32:48].rearrange("p (s c) -> p s c", s=32)
                for w_lo, ps in ((0, A2ps), (64, B2ps)):
                    for half, rhs in ((0, rhs_even), (1, rhs_odd)):
                        lhsT = Wl1[32 * half : 32 * half + 16, w_lo : w_lo + 64]
                        for sbh in (0, 1):
                            nc.tensor.matmul(
                                ps[64 * half : 64 * half + 64, 512 * sbh : 512 * sbh + 512],
                                lhsT,
                                rhs[:, 16 * sbh : 16 * sbh + 16, 0:32],
                            )
                # PE warm-up: dummy matmuls into a scratch bank keep the PE
                # busy through the otherwise-idle prologue window so the HAM
                # weight-load mode is already escalated when the L2 stream
                # starts (cold mode costs ~560ns vs ~330ns per mm).
                scratch = ab_pool.tile([128, 512], FP32, tag="warm", name="warm_t")[:]
                for _ in range(4):
                    nc.tensor.matmul(
                        scratch[0:64], Wl1[0:16, 0:64], statesQ[0:16, 0:512]
                    )
                # Evict A2 twice (duplicated pairs so the later broadcast add
                # keeps an innermost unit stride), folding in b1; B2 plain.
                # Pair-0's slice is evicted first in small ops so the first
                # h1 ADD can start ~2.5us earlier.
                dupview = A2dup.rearrange("p (c two) -> p two c", two=2)
                nc.scalar.activation(dupview[:, 0, 0:64], A2ps[:, 0:64], AF.Identity, bias=bias1)
                nc.scalar.activation(dupview[:, 1, 0:64], A2ps[:, 0:64], AF.Identity, bias=bias1)
                nc.vector.tensor_copy(B2s[:, 0:64], B2ps[:, 0:64])
                nc.scalar.activation(dupview[:, 0, 64:1024], A2ps[:, 64:1024], AF.Identity, bias=bias1)
                nc.scalar.activation(dupview[:, 1, 64:1024], A2ps[:, 64:1024], AF.Identity, bias=bias1)
                nc.vector.tensor_copy(B2s[:, 64:1024], B2ps[:, 64:1024])

            with (
                tc.tile_pool(name="work", bufs=8) as work_pool,
                tc.tile_pool(name="sigp", bufs=2) as sig_pool,
                tc.tile_pool(name="l2ps", bufs=2, space="PSUM") as l2_pool,
                tc.tile_pool(name="l3ps", bufs=2, space="PSUM") as l3_pool,
            ):
                # Software pipeline, 1 sb deep: L3(sb) (which needs S2(sb)'s
                # eviction) is emitted after L2(sb+1) so it never blocks the
                # next L2 in the strict-FIFO PE queue.
                h2_tiles = {}     # pair -> (h2a AP, h2b AP)
                psum3_tiles = {}  # pair -> psum3 AP
                sig_tiles = {}    # chunk -> sig AP
                dma_backlog = []  # (chunk, sig AP, next group) pending DMAs

                def emit_s1_add(pair):
                    h1pre = work_pool.tile([128, 2 * COLS], BF16, tag="h1pre", name="h1pre_t")[:]
                    a_in = (
                        A2dup[:, 128 * pair : 128 * pair + 128]
                        .rearrange("p (s i two) -> p s i two", s=2, two=2)
                        .unsqueeze(3)
                        .broadcast_to([128, 2, 32, 16, 2])
                    )
                    b_in = (
                        B2s[:, 64 * pair : 64 * pair + 64]
                        .rearrange("p (s jh jl) -> p s jh jl", s=2, jl=2)
                        .unsqueeze(2)
                        .broadcast_to([128, 2, 32, 16, 2])
                    )
                    h1pre_v = h1pre.rearrange(
                        "p (s i jh jl) -> p s i jh jl", s=2, i=32, jl=2
                    )
                    nc.vector.tensor_add(h1pre_v, a_in, b_in)
                    return h1pre

                def emit_s1_max(h1pre):
                    h1 = work_pool.tile([128, 2 * COLS], BF16, tag="h1", name="h1_t")[:]
                    nc.vector.tensor_scalar_max(h1, h1pre, 0.0)
                    return h1

                def emit_l2_pair(p, h1):
                    # Per-sb psums (bufs=2) keep the PE pipelined one sb
                    # ahead of the evictions.  Col half 512:1024 has its
                    # partition halves swapped (even block on 64:128) so all
                    # 4 quadrants run at once.
                    h2a = work_pool.tile([128, 2 * EV_SPLIT], BF16, tag="h2a", name="h2a_t")[:]
                    h2b = work_pool.tile([128, 2 * (COLS - EV_SPLIT)], BF16, tag="h2b", name="h2b_t")[:]
                    for k in (0, 1):
                        hk = h1[:, COLS * k : COLS * k + COLS]
                        psum2 = l2_pool.tile([128, 1024], FP32, tag="l2", name="psum2_t")[:]
                        nc.tensor.matmul(psum2[0:64, 0:512], W2q[0:64], hk[0:64, 0:512], tile_position=(0, 0))
                        nc.tensor.matmul(psum2[64:128, 0:512], W2q[64:128], hk[64:128, 0:512], tile_position=(64, 64))
                        nc.tensor.matmul(psum2[64:128, 512:1024], W2q[0:64], hk[0:64, 512:1024], tile_position=(0, 64))
                        nc.tensor.matmul(psum2[0:64, 512:1024], W2q[64:128], hk[64:128, 512:1024], tile_position=(64, 0))
                        nc.scalar.activation(
                            h2a[:, EV_SPLIT * k : EV_SPLIT * k + EV_SPLIT],
                            psum2[:, 0:EV_SPLIT], AF.Relu, bias=bias2,
                        )
                        nc.vector.tensor_scalar(
                            h2b[:, (COLS - EV_SPLIT) * k : (COLS - EV_SPLIT) * (k + 1)],
                            psum2[:, EV_SPLIT:COLS], bias2, 0.0, add, max_,
                        )
                    h2_tiles[p] = (h2a, h2b)

                def emit_l3(sb):
                    pair, k = divmod(sb, 2)
                    if k == 0:
                        psum3_tiles[pair] = l3_pool.tile([128, COLS], FP32, tag="l3", name="psum3_t")[:]
                    psum3 = psum3_tiles[pair]
                    h2a, h2b = h2_tiles[pair]
                    if k == 1:
                        del h2_tiles[pair]
                    # 8-tile L3: 256-col streams; psum3 col = 512p + 256k + q
                    # so col-group partner tiles (parities) land in different
                    # PSUM banks (concurrent same-bank writes from one
                    # col-group are a HW collision).
                    for g in range(4):
                        for p in (0, 1):
                            r = 64 * p if g < 2 else 64 * (1 - p)
                            c0 = 256 * g
                            if c0 >= EV_SPLIT:
                                cb = (COLS - EV_SPLIT) * k + c0 - EV_SPLIT
                                rhs = h2b[r : r + 64, cb : cb + 256]
                            else:
                                rhs = h2a[r : r + 64, EV_SPLIT * k + c0 : EV_SPLIT * k + c0 + 256]
                            nc.tensor.matmul(
                                psum3[32 * g : 32 * g + 32, 512 * p + 256 * k : 512 * p + 256 * k + 256],
                                W3q[r : r + 64],
                                rhs,
                                tile_position=(r, 32 * g),
                            )

                def emit_sigmoid_dma(pair):
                    t, u = divmod(pair, 4)
                    if u == 0:
                        sig_tiles[t] = sig_pool.tile([128, 4 * COLS], OUT_DT, tag="sig", name="sig_t")[:]
                    sig4 = sig_tiles[t]
                    psum3 = psum3_tiles.pop(pair)
                    nc.scalar.activation(
                        sig4[:, COLS * u : COLS * u + COLS], psum3, AF.Sigmoid, bias=bias3
                    )
                    if pair == NPAIR - 2:
                        # Final chunk, first half: drain cols 0:2048 while
                        # the last two pairs still compute.
                        for g in range(4):
                            eng = nc.sync if g < 2 else nc.scalar
                            eng.dma_start(
                                out=d_out[t, g][:, 0 : 2 * COLS],
                                in_=sig4[32 * g : 32 * g + F, 0 : 2 * COLS],
                            )
                    elif pair == NPAIR - 1:
                        # Final chunk, second half: 2 DMAs on the sync HWDGE
                        # ring, 2 on the scalar ring (idle after the last
                        # sigmoid) so descriptor gen + drain run in parallel.
                        for g in range(4):
                            eng = nc.sync if g < 2 else nc.scalar
                            eng.dma_start(
                                out=d_out[t, g][:, 2 * COLS : 4 * COLS],
                                in_=sig4[32 * g : 32 * g + F, 2 * COLS : 4 * COLS],
                            )
                    elif u == 3:
                        dma_backlog.append((t, sig4, 0))

                def emit_chunk_dma_piece():
                    # De-burst: one group-DMA of the previous chunk per pair.
                    if dma_backlog:
                        t, sig4, g = dma_backlog[0]
                        nc.sync.dma_start(
                            out=d_out[t, g], in_=sig4[32 * g : 32 * g + F, :]
                        )
                        if g == 3:
                            dma_backlog.pop(0)
                        else:
                            dma_backlog[0] = (t, sig4, g + 1)

                # Pair-level batching: both L2 groups back-to-back (W2 loads
                # amortized), then both L3 groups of the PREVIOUS pair (their
                # evictions finished a full cycle ago, so the strict-FIFO PE
                # queue never stalls and W3 streams back-to-back).  DVE order
                # per pair: ADDMAX(2p), ADDMAX(2p+1), ADD(p+1), MAX(p+1) —
                # evictions first so psum2 recycles early.
                h1_cur = emit_s1_max(emit_s1_add(0))
                for p in range(NPAIR):
                    emit_l2_pair(p, h1_cur)
                    if p >= 1:
                        emit_l3(2 * p - 2)
                        emit_l3(2 * p - 1)
                        emit_sigmoid_dma(p - 1)
                        emit_chunk_dma_piece()
                    if p + 1 < NPAIR:
                        h1_cur = emit_s1_max(emit_s1_add(p + 1))
                emit_l3(NSB - 2)
                emit_l3(NSB - 1)
                emit_sigmoid_dma(NPAIR - 1)

    nc.compile()
    input_names = ["statesQ", "Wl1", "W23", "biases"]
    return nc, input_names


def get_program():
    global _PROGRAM
    if _PROGRAM is None:
        _PROGRAM = _build_program()
    return _PROGRAM


def make_inputs(states, W1, b1, W2, b2, W3, b3):
    """Host-side prep: per-core statesQ + shared packed weights/biases."""
    states = np.asarray(states, np.float32)
    W1 = np.asarray(W1, np.float32)
    W2 = np.asarray(W2, np.float32)
    W3 = np.asarray(W3, np.float32)
    b1 = np.asarray(b1, np.float32)
    b2 = np.asarray(b2, np.float32)
    b3 = np.asarray(b3, np.float32)

    Wl1 = np.zeros((48, 128), NP_BF16)
    Wl1[0:16, 0:64] = W1[:D].astype(NP_BF16)
    Wl1[0:16, 64:128] = W1[D:].astype(NP_BF16)
    Wl1[32:48, 0:64] = W1[:D].astype(NP_BF16)
    Wl1[32:48, 64:128] = W1[D:].astype(NP_BF16)

    W23 = np.zeros((128, 96), NP_BF16)
    W23[0:64, 0:64] = W2.astype(NP_BF16)
    W23[64:128, 0:64] = W2.astype(NP_BF16)
    W23[0:64, 64:72] = W3.astype(NP_BF16)
    W23[64:128, 64:72] = W3.astype(NP_BF16)

    biases = np.zeros((128, 3), np.float32)
    biases[:, 0] = np.tile(b1, 2)
    biases[:, 1] = np.tile(b2, 2)
    biases[:, 2] = np.tile(np.concatenate([b3, np.zeros(24, np.float32)]), 4)

    shared = {"Wl1": Wl1, "W23": W23, "biases": biases}

    in_maps = []
    for c in range(NCORES):
        # statesT[d, 32*l + i] = states[c, l, i, d]
        statesT = states[c].reshape(L * N, D).T.astype(NP_BF16)
        sQs = np.zeros((16, 2048), NP_BF16)
        sQs[:, : 2048 - 32] = statesT[:, 32:]
        in_maps.append({"statesQ": statesT.copy(), "statesQs": sQs, **shared})
    return in_maps


def decode_output(raw):
    """Invert the device output layout -> [L, N, N, F] for one core.

    raw: [4, 4, F, 4096] fp16 = [chunk t, group g, f, col]; col =
    1024u + 512k + 256p + q with pair = 4t+u, sb = 2*pair+k,
    l = 2*sb + p = 16t + 4u + 2k + p; pair-col = 256g + q = 32i + j.
    """
    ov = raw.reshape(4, 4, F, 4, 2, 2, 8, 32)        # [t, g, f, u, p, k, qi, j]
    ov = ov.transpose(0, 3, 5, 4, 1, 6, 7, 2)        # [t, u, k, p, g, qi, j, f]
    return np.ascontiguousarray(ov.reshape(L, N, N, F).astype(np.float32))


def _ensure_ntff_hook():
    """Best-effort shim for the missing antenv.axon_hooks module so
    run_bass_kernel_spmd(trace=True) can capture NTFF profiles under axon."""
    import types

    try:
        from antenv.axon_hooks import get_axon_ntff_profile_hook  # noqa: F401
        return
    except ImportError:
        pass
    try:
        if "/root/.axon_site" not in sys.path:
            sys.path.insert(0, "/root/.axon_site")
        from trn_agent_boot.trn_boot import _ntff_profile_via_ctypes

        hook = _ntff_profile_via_ctypes("/opt/axon/libaxon_pjrt.so")
        import antenv

        mod = types.ModuleType("antenv.axon_hooks")
        mod._hook = hook
        mod.set_axon_ntff_profile_hook = lambda h: setattr(mod, "_hook", h)
        mod.get_axon_ntff_profile_hook = lambda: mod._hook
        sys.modules["antenv.axon_hooks"] = mod
        antenv.axon_hooks = mod
    except Exception as e:  # tracing is optional; never break the run
        print(f"ntff hook shim failed: {e}", file=sys.stderr)


def kernel(states, W1, b1, W2, b2, W3, b3):
    global LAST_RESULT
    nc, _ = get_program()
    if os.environ.get("KERNEL_TRACE"):
        _ensure_ntff_hook()
    in_maps = make_inputs(states, W1, b1, W2, b2, W3, b3)
    res = run_bass_kernel_spmd(
        nc,
        in_maps,
        core_ids=list(range(NCORES)),
        trace=bool(os.environ.get("KERNEL_TRACE")),
    )
    LAST_RESULT = res
    out = np.empty((B, L, N, N, F), np.float32)
    for c in range(NCORES):
        out[c] = decode_output(res.results[c]["out"])
    return out


# revision 35
# speedup vs baseline: 1.5113x; 1.0153x over previous
"""Trainium2 Bass kernel for pairwise-MLP GNN message passing.

Computation (per batch b, position l):
    x[i,j] = concat(states[l,i], states[l,j])           # [N,N,2D]
    out    = sigmoid(MLP(x))                            # [N,N,8], MLP: 32->64->64->8

Factorization used on device: the first linear layer splits into
A = states @ W1[:D] + b1 and B = states @ W1[D:], so
h1[i,j] = relu(A[i] + B[j]) — the N^2 expansion happens as a cheap
broadcast add on the vector engine instead of an N^2-row matmul.

Sharding: data-parallel over batch, core c <- batch c (8 cores, B=8).

Device layout (per core, L=64 l-blocks, 2 l-blocks = 1 "sb" superblock,
2 sbs = 1 "pair", 4 pairs = 1 DMA chunk):
  - features live on partitions: partitions 0:64 = even l-block of the sb,
    64:128 = odd l-block (via a host-side shifted copy of states^T feeding
    block-diagonal-packed matmuls).
  - pair columns col = 32*i + j, 1024 per l-block.
  - L2 runs as 4 concurrent 64x64x512 tile_position matmuls into one
    2-bank psum2; eviction split 768 cols on ScalarE / 256 on VectorE.
  - L3 runs as 8 concurrent 64x32x256 tile_position matmuls; psum3
    partition group 32g..32g+32 holds pair-column chunk g.
  - Sigmoid evicts psum3 as fp16 into a [128, 4096] tile covering 4
    pairs; one gathered DMA per chunk moves the 32 useful partitions.
"""

import os
import sys

import numpy as np

for _p in ("/opt/trn_rl_repo", "/root/.axon_site/_ro/trn_rl_repo"):
    if os.path.isdir(_p) and _p not in sys.path:
        sys.path.insert(0, _p)

from concourse import bacc, mybir, tile
from concourse.bass_utils import run_bass_kernel_spmd

B, L, N, D = 8, 64, 32, 16
H = 64            # hidden width (h1 and h2)
F = 8             # out_dim
NCORES = 8
NSB = L // 2      # 32 superblocks per core
NPAIR = NSB // 2  # 16 pairs per core
COLS = N * N      # 1024 pair columns per l-block
EV_SPLIT = 768    # h2 eviction: cols [0, EV_SPLIT) on ScalarE, rest on VectorE

FP32 = mybir.dt.float32
FP16 = mybir.dt.float16
OUT_DT = mybir.dt.float16  # device output dtype
BF16 = mybir.dt.bfloat16
NP_BF16 = mybir.dt.np(BF16)

_PROGRAM = None  # (nc, input_names)
LAST_RESULT = None  # BassKernelResults of the most recent kernel() call


def _build_program():
    nc = bacc.Bacc("TRN2", target_bir_lowering=False, debug=False)

    d_statesQ = nc.dram_tensor("statesQ", [16, 2048], BF16, kind="ExternalInput").ap()
    d_statesQs = nc.dram_tensor("statesQs", [16, 2048], BF16, kind="ExternalInput").ap()
    d_Wl1 = nc.dram_tensor("Wl1", [48, 128], BF16, kind="ExternalInput").ap()
    d_W23 = nc.dram_tensor("W23", [128, 96], BF16, kind="ExternalInput").ap()
    d_biases = nc.dram_tensor("biases", [128, 3], FP32, kind="ExternalInput").ap()
    # chunk t = pairs 4t..4t+4; [t, group g, feature f, col]
    d_out = nc.dram_tensor(
        "out", [NPAIR // 4, 4, F, 4 * COLS], OUT_DT, kind="ExternalOutput"
    ).ap()

    add = mybir.AluOpType.add
    max_ = mybir.AluOpType.max
    AF = mybir.ActivationFunctionType

    with tile.TileContext(nc) as tc:
        with tc.tile_pool(name="const", bufs=1) as const_pool:
            statesQ = const_pool.tile([64, 2048], BF16, name="statesQ_t")[:]
            Wl1 = const_pool.tile([48, 128], BF16, name="Wl1_t")[:]
            W23 = const_pool.tile([128, 96], BF16, name="W23_t")[:]
            biases = const_pool.tile([128, 3], FP32, name="biases_t")[:]
            W2q = W23[:, 0:64]
            W3q = W23[:, 64:96]
            bias1 = biases[:, 0:1]
            bias2 = biases[:, 1:2]
            bias3 = biases[:, 2:3]
            A2dup = const_pool.tile([128, 2 * COLS], BF16, name="A2dup_t")[:]
            B2s = const_pool.tile([128, COLS], BF16, name="B2s_t")[:]

            nc.sync.dma_start(out=statesQ[0:16], in_=d_statesQ)
            nc.sync.dma_start(out=statesQ[32:48], in_=d_statesQs)
            nc.sync.dma_start(out=Wl1, in_=d_Wl1)
            nc.sync.dma_start(out=W23, in_=d_W23)
            nc.sync.dma_start(out=biases, in_=d_biases)

            # ---- Layer 1: A2/B2 = per-agent halves of the first linear layer.
            # A2[p, 32*sb + i]: p<64 -> even l-block (2sb), p>=64 -> odd (2sb+1)
            # via the shifted rows 32:48 of statesQ.
            with tc.tile_pool(name="abps", bufs=1, space="PSUM") as ab_pool:
                A2ps = ab_pool.tile([128, COLS], FP32, tag="a2", name="A2ps_t")[:]
                B2ps = ab_pool.tile([128, COLS], FP32, tag="b2", name="B2ps_t")[:]
                rhs_even = statesQ[0:16].rearrange("p (s c) -> p s c", s=32)
                rhs_odd = statesQ[32:48].rearrange("p (s c) -> p s c", s=32)
                for w_lo, ps in ((0, A2ps), (64, B2ps)):
                    for half, rhs in ((0, rhs_even), (1, rhs_odd)):
                        lhsT = Wl1[32 * half : 32 * half + 16, w_lo : w_lo + 64]
                        for sbh in (0, 1):
                            nc.tensor.matmul(
                                ps[64 * half : 64 * half + 64, 512 * sbh : 512 * sbh + 512],
                                lhsT,
                                rhs[:, 16 * sbh : 16 * sbh + 16, 0:32],
                            )
                # Evict A2 twice (duplicated pairs so the later broadcast add
                # keeps an innermost unit stride), folding in b1; B2 plain.
                # Pair-0's slice is evicted first in small ops so the first
                # h1 ADD can start ~2.5us earlier.
                dupview = A2dup.rearrange("p (c two) -> p two c", two=2)
                nc.scalar.activation(dupview[:, 0, 0:64], A2ps[:, 0:64], AF.Identity, bias=bias1)
                nc.scalar.activation(dupview[:, 1, 0:64], A2ps[:, 0:64], AF.Identity, bias=bias1)
                nc.vector.tensor_copy(B2s[:, 0:64], B2ps[:, 0:64])
                nc.scalar.activation(dupview[:, 0, 64:1024], A2ps[:, 64:1024], AF.Identity, bias=bias1)
                nc.scalar.activation(dupview[:, 1, 64:1024], A2ps[:, 64:1024], AF.Identity, bias=bias1)
                nc.vector.tensor_copy(B2s[:, 64:1024], B2ps[:, 64:1024])

            with (
                tc.tile_pool(name="work", bufs=4) as work_pool,
                tc.tile_pool(name="sigp", bufs=2) as sig_pool,
                tc.tile_pool(name="l2ps", bufs=2, space="PSUM") as l2_pool,
                tc.tile_pool(name="l3ps", bufs=2, space="PSUM") as l3_pool,
            ):
                # Software pipeline, 1 sb deep: L3(sb) (which needs S2(sb)'s
                # eviction) is emitted after L2(sb+1) so it never blocks the
                # next L2 in the strict-FIFO PE queue.
                h2_tiles = {}     # pair -> (h2a AP, h2b AP)
                psum3_tiles = {}  # pair -> psum3 AP
                sig_tiles = {}    # chunk -> sig AP
                dma_backlog = []  # (chunk, sig AP, next group) pending DMAs

                def emit_s1_add(pair):
                    h1pre = work_pool.tile([128, 2 * COLS], BF16, tag="h1pre", name="h1pre_t")[:]
                    a_in = (
                        A2dup[:, 128 * pair : 128 * pair + 128]
                        .rearrange("p (s i two) -> p s i two", s=2, two=2)
                        .unsqueeze(3)
                        .broadcast_to([128, 2, 32, 16, 2])
                    )
                    b_in = (
                        B2s[:, 64 * pair : 64 * pair + 64]
                        .rearrange("p (s jh jl) -> p s jh jl", s=2, jl=2)
                        .unsqueeze(2)
                        .broadcast_to([128, 2, 32, 16, 2])
                    )
                    h1pre_v = h1pre.rearrange(
                        "p (s i jh jl) -> p s i jh jl", s=2, i=32, jl=2
                    )
                    nc.vector.tensor_add(h1pre_v, a_in, b_in)
                    return h1pre

                def emit_s1_max(h1pre):
                    h1 = work_pool.tile([128, 2 * COLS], BF16, tag="h1", name="h1_t")[:]
                    nc.vector.tensor_scalar_max(h1, h1pre, 0.0)
                    return h1

                def emit_l2_pair(p, h1):
                    # Per-sb psums (bufs=2) keep the PE pipelined one sb
                    # ahead of the evictions.  Col half 512:1024 has its
                    # partition halves swapped (even block on 64:128) so all
                    # 4 quadrants run at once.
                    h2a = work_pool.tile([128, 2 * EV_SPLIT], BF16, tag="h2a", name="h2a_t")[:]
                    h2b = work_pool.tile([128, 2 * (COLS - EV_SPLIT)], BF16, tag="h2b", name="h2b_t")[:]
                    for k in (0, 1):
                        hk = h1[:, COLS * k : COLS * k + COLS]
                        psum2 = l2_pool.tile([128, 1024], FP32, tag="l2", name="psum2_t")[:]
                        nc.tensor.matmul(psum2[0:64, 0:512], W2q[0:64], hk[0:64, 0:512], tile_position=(0, 0))
                        nc.tensor.matmul(psum2[64:128, 0:512], W2q[64:128], hk[64:128, 0:512], tile_position=(64, 64))
                        nc.tensor.matmul(psum2[64:128, 512:1024], W2q[0:64], hk[0:64, 512:1024], tile_position=(0, 64))
                        nc.tensor.matmul(psum2[0:64, 512:1024], W2q[64:128], hk[64:128, 512:1024], tile_position=(64, 0))
                        nc.scalar.activation(
                            h2a[:, EV_SPLIT * k : EV_SPLIT * k + EV_SPLIT],
                            psum2[:, 0:EV_SPLIT], AF.Relu, bias=bias2,
                        )
                        nc.vector.tensor_scalar(
                            h2b[:, (COLS - EV_SPLIT) * k : (COLS - EV_SPLIT) * (k + 1)],
                            psum2[:, EV_SPLIT:COLS], bias2, 0.0, add, max_,
                        )
                    h2_tiles[p] = (h2a, h2b)

                def emit_l3(sb):
                    pair, k = divmod(sb, 2)
                    if k == 0:
                        psum3_tiles[pair] = l3_pool.tile([128, COLS], FP32, tag="l3", name="psum3_t")[:]
                    psum3 = psum3_tiles[pair]
                    h2a, h2b = h2_tiles[pair]
                    if k == 1:
                        del h2_tiles[pair]
                    # 8-tile L3: 256-col streams; psum3 col = 512p + 256k + q
                    # so col-group partner tiles (parities) land in different
                    # PSUM banks (concurrent same-bank writes from one
                    # col-group are a HW collision).
                    for g in range(4):
                        for p in (0, 1):
                            r = 64 * p if g < 2 else 64 * (1 - p)
                            c0 = 256 * g
                            if c0 >= EV_SPLIT:
                                cb = (COLS - EV_SPLIT) * k + c0 - EV_SPLIT
                                rhs = h2b[r : r + 64, cb : cb + 256]
                            else:
                                rhs = h2a[r : r + 64, EV_SPLIT * k + c0 : EV_SPLIT * k + c0 + 256]
                            nc.tensor.matmul(
                                psum3[32 * g : 32 * g + 32, 512 * p + 256 * k : 512 * p + 256 * k + 256],
                                W3q[r : r + 64],
                                rhs,
                                tile_position=(r, 32 * g),
                            )

                def emit_sigmoid_dma(pair):
                    t, u = divmod(pair, 4)
                    if u == 0:
                        sig_tiles[t] = sig_pool.tile([128, 4 * COLS], OUT_DT, tag="sig", name="sig_t")[:]
                    sig4 = sig_tiles[t]
                    psum3 = psum3_tiles.pop(pair)
                    nc.scalar.activation(
                        sig4[:, COLS * u : COLS * u + COLS], psum3, AF.Sigmoid, bias=bias3
                    )
                    if pair == NPAIR - 2:
                        # Final chunk, first half: drain cols 0:2048 while
                        # the last two pairs still compute.
                        for g in range(4):
                            eng = nc.sync if g < 2 else nc.scalar
                            eng.dma_start(
                                out=d_out[t, g][:, 0 : 2 * COLS],
                                in_=sig4[32 * g : 32 * g + F, 0 : 2 * COLS],
                            )
                    elif pair == NPAIR - 1:
                        # Final chunk, second half: 2 DMAs on the sync HWDGE
                        # ring, 2 on the scalar ring (idle after the last
                        # sigmoid) so descriptor gen + drain run in parallel.
                        for g in range(4):
                            eng = nc.sync if g < 2 else nc.scalar
                            eng.dma_start(
                                out=d_out[t, g][:, 2 * COLS : 4 * COLS],
                                in_=sig4[32 * g : 32 * g + F, 2 * COLS : 4 * COLS],
                            )
                    elif u == 3:
                        dma_backlog.append((t, sig4, 0))

                def emit_chunk_dma_piece():
                    # De-burst: one group-DMA of the previous chunk per pair.
                    if dma_backlog:
                        t, sig4, g = dma_backlog[0]
                        nc.sync.dma_start(
                            out=d_out[t, g], in_=sig4[32 * g : 32 * g + F, :]
                        )
                        if g == 3:
                            dma_backlog.pop(0)
                        else:
                            dma_backlog[0] = (t, sig4, g + 1)

                # Pair-level batching: both L2 groups back-to-back (W2 loads
                # amortized), then both L3 groups of the PREVIOUS pair (their
                # evictions finished a full cycle ago, so the strict-FIFO PE
                # queue never stalls and W3 streams back-to-back).  DVE order
                # per pair: ADDMAX(2p), ADDMAX(2p+1), ADD(p+1), MAX(p+1) —
                # evictions first so psum2 recycles early.
                h1_cur = emit_s1_max(emit_s1_add(0))
                for p in range(NPAIR):
                    emit_l2_pair(p, h1_cur)
                    if p >= 1:
                        emit_l3(2 * p - 2)
                        emit_l3(2 * p - 1)
                        emit_sigmoid_dma(p - 1)
                        emit_chunk_dma_piece()
                    if p + 1 < NPAIR:
                        h1_cur = emit_s1_max(emit_s1_add(p + 1))
                emit_l3(NSB - 2)
                emit_l3(NSB - 1)
                emit_sigmoid_dma(NPAIR - 1)

    nc.compile()
    input_names = ["statesQ", "Wl1", "W23", "biases"]
    return nc, input_names


def get_program():
    global _PROGRAM
    if _PROGRAM is None:
        _PROGRAM = _build_program()
    return _PROGRAM


def make_inputs(states, W1, b1, W2, b2, W3, b3):
    """Host-side prep: per-core statesQ + shared packed weights/biases."""
    states = np.asarray(states, np.float32)
    W1 = np.asarray(W1, np.float32)
    W2 = np.asarray(W2, np.float32)
    W3 = np.asarray(W3, np.float32)
    b1 = np.asarray(b1, np.float32)
    b2 = np.asarray(b2, np.float32)
    b3 = np.asarray(b3, np.float32)

    Wl1 = np.zeros((48, 128), NP_BF16)
    Wl1[0:16, 0:64] = W1[:D].astype(NP_BF16)
    Wl1[0:16, 64:128] = W1[D:].astype(NP_BF16)
    Wl1[32:48, 0:64] = W1[:D].astype(NP_BF16)
    Wl1[32:48, 64:128] = W1[D:].astype(NP_BF16)

    W23 = np.zeros((128, 96), NP_BF16)
    W23[0:64, 0:64] = W2.astype(NP_BF16)
    W23[64:128, 0:64] = W2.astype(NP_BF16)
    W23[0:64, 64:72] = W3.astype(NP_BF16)
    W23[64:128, 64:72] = W3.astype(NP_BF16)

    biases = np.zeros((128, 3), np.float32)
    biases[:, 0] = np.tile(b1, 2)
    biases[:, 1] = np.tile(b2, 2)
    biases[:, 2] = np.tile(np.concatenate([b3, np.zeros(24, np.float32)]), 4)

    shared = {"Wl1": Wl1, "W23": W23, "biases": biases}

    in_maps = []
    for c in range(NCORES):
        # statesT[d, 32*l + i] = states[c, l, i, d]
        statesT = states[c].reshape(L * N, D).T.astype(NP_BF16)
        sQs = np.zeros((16, 2048), NP_BF16)
        sQs[:, : 2048 - 32] = statesT[:, 32:]
        in_maps.append({"statesQ": statesT.copy(), "statesQs": sQs, **shared})
    return in_maps


def decode_output(raw):
    """Invert the device output layout -> [L, N, N, F] for one core.

    raw: [4, 4, F, 4096] fp16 = [chunk t, group g, f, col]; col =
    1024u + 512k + 256p + q with pair = 4t+u, sb = 2*pair+k,
    l = 2*sb + p = 16t + 4u + 2k + p; pair-col = 256g + q = 32i + j.
    """
    ov = raw.reshape(4, 4, F, 4, 2, 2, 8, 32)        # [t, g, f, u, p, k, qi, j]
    ov = ov.transpose(0, 3, 5, 4, 1, 6, 7, 2)        # [t, u, k, p, g, qi, j, f]
    return np.ascontiguousarray(ov.reshape(L, N, N, F).astype(np.float32))


def _ensure_ntff_hook():
    """Best-effort shim for the missing antenv.axon_hooks module so
    run_bass_kernel_spmd(trace=True) can capture NTFF profiles under axon."""
    import types

    try:
        from antenv.axon_hooks import get_axon_ntff_profile_hook  # noqa: F401
        return
    except ImportError:
        pass
    try:
        if "/root/.axon_site" not in sys.path:
            sys.path.insert(0, "/root/.axon_site")
        from trn_agent_boot.trn_boot import _ntff_profile_via_ctypes

        hook = _ntff_profile_via_ctypes("/opt/axon/libaxon_pjrt.so")
        import antenv

        mod = types.ModuleType("antenv.axon_hooks")
        mod._hook = hook
        mod.set_axon_ntff_profile_hook = lambda h: setattr(mod, "_hook", h)
        mod.get_axon_ntff_profile_hook = lambda: mod._hook
        sys.modules["antenv.axon_hooks"] = mod
        antenv.axon_hooks = mod
    except Exception as e:  # tracing is optional; never break the run
        print(f"ntff hook shim failed: {e}", file=sys.stderr)


def kernel(states, W1, b1, W2, b2, W3, b3):
    global LAST_RESULT
    nc, _ = get_program()
    if os.environ.get("KERNEL_TRACE"):
        _ensure_ntff_hook()
    in_maps = make_inputs(states, W1, b1, W2, b2, W3, b3)
    res = run_bass_kernel_spmd(
        nc,
        in_maps,
        core_ids=list(range(NCORES)),
        trace=bool(os.environ.get("KERNEL_TRACE")),
    )
    LAST_RESULT = res
    out = np.empty((B, L, N, N, F), np.float32)
    for c in range(NCORES):
        out[c] = decode_output(res.results[c]["out"])
    return out
